# revision 3
# baseline (speedup 1.0000x reference)
"""Bass/Tile kernel for LocalWindowMultiHeadAttention on 8 trn2 cores.

v2: blocked transposed-score formulation. Per core (16 query rows):
  scores computed directly in [kpix, qx] orientation as S^T tiles
  [(4 krows x 32 kx), 26 qx] via K=32 matmuls (QE/QO even/odd-head channel
  masking), killing all P transposes. exp on ACT per half-row (4 heads x
  2 tiles x 128 qx), band-mask multiply on DVE/Pool, attn@V contracts the
  masked P tiles against V^T tiles (PE-transposed from the V projection)
  with a ones-column matmul for Z. Per-block normalize (scalar_tensor_tensor
  with 1/Z broadcast) into block-major attnN, per-block PE transposes
  reassemble [hd, qx], then the Wo projection.
"""

import sys
import numpy as np
import ml_dtypes
from contextlib import ExitStack

sys.path.insert(0, "/opt/trn_rl_repo")

import concourse.bass as bass
import concourse.mybir as mybir
import concourse.tile as tile
from concourse import bacc
from concourse.masks import make_identity
from concourse import bass_utils

BF16 = mybir.dt.bfloat16
F32 = mybir.dt.float32
ALU = mybir.AluOpType
AF = mybir.ActivationFunctionType

C = 128
NH = 8
HD = 16
R = 3
WIN = 7
H = W = 128
RPC = 16                  # query rows per core
KR = 23                   # padded k-rows per core (22 real + 1 pad)
KW = 136                  # padded k-cols (134 real + 2 pad)
NKPIX = KR * KW           # 3128
NQ = RPC * W              # 2048
SCALE = 1.0 / 4.0
NB = 5                    # qx blocks per row: 26,26,26,26,24
BW = [26, 26, 26, 26, 24]
BO = [0, 26, 52, 78, 104]
NS = 20                   # V^T tile starts (krow s..s+4)

_CACHE = {}


def build_nc():
    nc = bacc.Bacc()
    xT = nc.dram_tensor("xT", [C, NKPIX], BF16, kind="ExternalInput")
    wqe = nc.dram_tensor("wqe", [C, C], BF16, kind="ExternalInput")
    wqo = nc.dram_tensor("wqo", [C, C], BF16, kind="ExternalInput")
    wk = nc.dram_tensor("wk", [C, C], BF16, kind="ExternalInput")
    wv = nc.dram_tensor("wv", [C, C], BF16, kind="ExternalInput")
    wo = nc.dram_tensor("wo", [C, C], BF16, kind="ExternalInput")
    mask = nc.dram_tensor("mask", [C, 2 * 128], BF16, kind="ExternalInput")
    yT = nc.dram_tensor("yT", [C, NQ], F32, kind="ExternalOutput")

    with tile.TileContext(nc) as tc, ExitStack() as ctx:
        const = ctx.enter_context(tc.tile_pool(name="const", bufs=1))
        sb = ctx.enter_context(tc.tile_pool(name="sb", bufs=1))
        pbuf = ctx.enter_context(tc.tile_pool(name="pbuf", bufs=2))
        wrk = ctx.enter_context(tc.tile_pool(name="wrk", bufs=2))
        # PSUM: sc 2x2 banks, av 2 banks, fin 1 bank
        ps_sc = ctx.enter_context(tc.tile_pool(name="sc", bufs=2, space="PSUM"))
        ps_av = ctx.enter_context(tc.tile_pool(name="av", bufs=1, space="PSUM"))
        ps_fin = ctx.enter_context(tc.tile_pool(name="fin", bufs=1, space="PSUM"))

        ident = const.tile([128, 128], BF16)
        make_identity(nc, ident[:])
        ones_sb = const.tile([128, 1], BF16)
        nc.vector.memset(ones_sb[:], 1.0)

        xT_sb = const.tile([C, KR, KW], BF16)
        nc.sync.dma_start(xT_sb[:], xT[:].rearrange("c (r x) -> c r x", x=KW))
        wqe_sb = const.tile([C, C], BF16)
        nc.sync.dma_start(wqe_sb[:], wqe[:])
        wqo_sb = const.tile([C, C], BF16)
        nc.sync.dma_start(wqo_sb[:], wqo[:])
        wk_sb = const.tile([C, C], BF16)
        nc.sync.dma_start(wk_sb[:], wk[:])
        wv_sb = const.tile([C, C], BF16)
        nc.sync.dma_start(wv_sb[:], wv[:])
        wo_sb = const.tile([C, C], BF16)
        nc.sync.dma_start(wo_sb[:], wo[:])
        mask_sb = const.tile([C, 2, 128], BF16)
        nc.sync.dma_start(mask_sb[:], mask[:].rearrange("c (t q) -> c t q", q=128))

        KS = const.tile([C, KR, KW], BF16)
        VS = const.tile([C, KR, KW], BF16)
        QE = const.tile([C, RPC, W], BF16)
        QO = const.tile([C, RPC, W], BF16)
        VT = const.tile([128, NS, NB, 128], BF16)

        xflat = xT_sb[:].rearrange("c r x -> c (r x)")

        # ---- projections: K, V over all kpix; Q (even/odd) over centers ----
        # chunks of 1024 cols through the sc pool (2 banks each, bufs=2)
        def proj_copy(dst_flat, w_sb, src_flat, n0, n, eng):
            t = ps_sc.tile([128, 1024], F32, tag="sc")
            h1 = min(512, n)
            nc.tensor.matmul(t[:, 0:h1], w_sb[:], src_flat[:, n0:n0 + h1],
                             start=True, stop=True)
            if n > 512:
                nc.tensor.matmul(t[:, 512:512 + n - 512], w_sb[:],
                                 src_flat[:, n0 + 512:n0 + n],
                                 start=True, stop=True)
            if eng == "act":
                nc.scalar.copy(dst_flat[:, n0:n0 + n], t[:, 0:n])
            else:
                nc.vector.tensor_copy(dst_flat[:, n0:n0 + n], t[:, 0:n])

        KSf = KS[:].rearrange("c r x -> c (r x)")
        VSf = VS[:].rearrange("c r x -> c (r x)")
        QEf = QE[:].rearrange("c r x -> c (r x)")
        QOf = QO[:].rearrange("c r x -> c (r x)")
        # interleave K and Q first (needed by scores of early rows), then V
        proj_copy(KSf, wk_sb, xflat, 0, 1024, "dve")
        qsrc = xT_sb[:, 3:3 + RPC, 3:3 + W].rearrange("c r x -> c (r x)")
        proj_copy(QEf, wqe_sb, qsrc, 0, 1024, "act")
        proj_copy(QOf, wqo_sb, qsrc, 0, 1024, "act")
        proj_copy(KSf, wk_sb, xflat, 1024, 1024, "dve")
        proj_copy(QEf, wqe_sb, qsrc, 1024, 1024, "act")
        proj_copy(QOf, wqo_sb, qsrc, 1024, 1024, "act")
        proj_copy(KSf, wk_sb, xflat, 2048, 1024, "dve")
        proj_copy(KSf, wk_sb, xflat, 3072, NKPIX - 3072, "dve")
        for j in range(4):
            n0 = 1024 * j
            n = min(1024, NKPIX - n0)
            proj_copy(VSf, wv_sb, xflat, n0, n, "act" if j % 2 else "dve")

        # ---- V^T tiles via PE transpose of VS ----
        def emit_vt(s):
            vtp = ps_fin.tile([128, NB, 128], BF16, tag="vt")
            for b in range(NB):
                nc.tensor.transpose(vtp[:, b, :],
                                    VS[:, s:s + 4, BO[b]:BO[b] + 32]
                                    .rearrange("c r x -> c (r x)"),
                                    ident[:])
            nc.vector.tensor_copy(VT[:, s, :, :], vtp[:])

        for s in range(4):
            emit_vt(s)

        mask_bc = mask_sb[:].rearrange("c t q -> c () t q").broadcast_to(
            [C, 4, 2, 128])

        # ---- main pipeline over rows ----
        sc_tiles = {}
        PM_tiles = {}
        av_tiles = {}
        nb_tiles = {}
        atS_tiles = {}

        def emit_scores(r, half):
            t = ps_sc.tile([128, 1024], F32, tag="sc")
            sc_tiles[(r, half)] = t
            tv = t[:].rearrange("p (h t q) -> p h t q", h=4, t=2)
            for g01 in range(2):
                g = 2 * half + g01
                pb = 32 * g
                for b in range(NB):
                    for tt in range(2):
                        lhs = KS[pb:pb + 32, r + 4 * tt:r + 4 * tt + 4,
                                 BO[b]:BO[b] + 32]
                        for par in range(2):
                            h4 = 2 * g01 + par
                            qsrc_ = QE if par == 0 else QO
                            rhs = qsrc_[pb:pb + 32, r, BO[b]:BO[b] + BW[b]]
                            nc.tensor.matmul(
                                tv[:, h4, tt, BO[b]:BO[b] + BW[b]],
                                lhs, rhs, start=True, stop=True,
                                tile_position=(pb, 0))

        def emit_exp_mask(r, half):
            t = sc_tiles.pop((r, half))
            P = pbuf.tile([128, 4, 2, 128], BF16, tag=f"P{half}")
            nc.scalar.activation(P[:].rearrange("p h t q -> p (h t q)"),
                                 t[:], AF.Exp, scale=SCALE)
            PM = pbuf.tile([128, 4, 2, 128], BF16, tag=f"PM{half}")
            PM_tiles[(r, half)] = PM
            if half == 0:
                nc.vector.tensor_mul(PM[:], P[:], mask_bc)
            else:
                nc.gpsimd.tensor_mul(PM[:], P[:], mask_bc)

        def emit_attnv(r):
            av = ps_av.tile([26, NB * 128 + NB * 8], F32, tag="av")
            av_tiles[r] = av
            attpD = av[:, 0:NB * 128].rearrange("p (b h d) -> p b h d",
                                                b=NB, h=NH)
            for b in range(NB):
                for h in range(NH):
                    half, h4 = divmod(h, 4)
                    PM = PM_tiles[(r, half)]
                    zc = NB * 128 + b * NH + h
                    for tt in range(2):
                        lhs = PM[:, h4, tt, BO[b]:BO[b] + BW[b]]
                        nc.tensor.matmul(
                            attpD[0:BW[b], b, h, :], lhs,
                            VT[:, r + 4 * tt, b, HD * h:HD * h + HD],
                            start=(tt == 0), stop=(tt == 1))
                        nc.tensor.matmul(
                            av[0:BW[b], zc:zc + 1], lhs, ones_sb[:],
                            start=(tt == 0), stop=(tt == 1))
            PM_tiles.pop((r, 0))
            PM_tiles.pop((r, 1))

        def emit_norm(r):
            av = av_tiles.pop(r)
            attpD = av[:, 0:NB * 128].rearrange("p (b h d) -> p b h d",
                                                b=NB, h=NH)
            Zp = av[:, NB * 128:].rearrange("p (b h) -> p b h", b=NB)
            rz = wrk.tile([26, NB, NH], F32, tag="rz")
            nc.vector.reciprocal(rz[:], Zp)
            nb_t = wrk.tile([26, NB, NH, HD], BF16, tag="nb")
            nb_tiles[r] = nb_t
            rzb = rz[:].rearrange("p b h -> p b h ()").broadcast_to(
                [26, NB, NH, HD])
            nc.vector.scalar_tensor_tensor(nb_t[:], attpD, 1.0, rzb,
                                           ALU.mult, ALU.mult)

        def emit_final(r):
            nb_t = nb_tiles.pop(r)
            atp = ps_fin.tile([128, 128], BF16, tag="atp")
            for b in range(NB):
                nc.tensor.transpose(
                    atp[:, BO[b]:BO[b] + BW[b]],
                    nb_t[0:BW[b], b, :, :].rearrange("p h d -> p (h d)"),
                    ident[0:BW[b], 0:BW[b]])
            atS = wrk.tile([128, 128], BF16, tag="atS")
            nc.vector.tensor_copy(atS[:], atp[:])
            yp = ps_fin.tile([128, 128], F32, tag="yp")
            nc.tensor.matmul(yp[:], wo_sb[:], atS[:], start=True, stop=True)
            yS = wrk.tile([128, 128], F32, tag="yS")
            nc.scalar.copy(yS[:], yp[:])
            nc.sync.dma_start(yT[:, W * r:W * (r + 1)], yS[:])

        for r in range(RPC + 2):
            if r < RPC:
                emit_scores(r, 0)
                emit_exp_mask(r, 0)
                emit_scores(r, 1)
                emit_exp_mask(r, 1)
                if r + 4 < NS:
                    emit_vt(r + 4)
            if 1 <= r <= RPC:
                emit_attnv(r - 1)
                emit_norm(r - 1)
            if r >= 2:
                emit_final(r - 2)
    nc.compile()
    return nc


def _get_nc():
    if "nc" not in _CACHE:
        _CACHE["nc"] = build_nc()
    return _CACHE["nc"]


def _host_mask():
    m = np.zeros((128, 2, 128), np.float32)
    p = np.arange(128)
    kxl = p % 32
    qx = np.arange(128)
    b = np.minimum(qx // 26, 4)
    qxl = qx - 26 * b
    band = (qxl[None, :] <= kxl[:, None]) & (kxl[:, None] <= qxl[None, :] + 6)
    m[:, 0, :] = band
    m[:, 1, :] = band & (p < 96)[:, None]
    return np.ascontiguousarray(m.reshape(128, 256)).astype(ml_dtypes.bfloat16)


def _kernel_bass(x, Wq, bq, Wk, bk, Wv, bv, Wo, bo):
    x = np.asarray(x, np.float32)
    Wq, Wk, Wv, Wo = (np.asarray(w, np.float32) for w in (Wq, Wk, Wv, Wo))

    xp = np.pad(x, ((0, 0), (R, R), (R, R), (0, 0)), mode="reflect")[0]
    bf = ml_dtypes.bfloat16
    ch = np.arange(C)
    even = ((ch // HD) % 2 == 0).astype(np.float32)
    wqe_t = np.ascontiguousarray(Wq.T * even[None, :]).astype(bf)
    wqo_t = np.ascontiguousarray(Wq.T * (1.0 - even)[None, :]).astype(bf)
    wk_t = np.ascontiguousarray(Wk.T).astype(bf)
    wv_t = np.ascontiguousarray(Wv.T).astype(bf)
    wo_t = np.ascontiguousarray(Wo.T).astype(bf)
    maskc = _host_mask()

    in_maps = []
    for i in range(8):
        xs = np.zeros((KR, KW, C), np.float32)
        nrows = min(KR, 134 - RPC * i)
        xs[:nrows, :134] = xp[RPC * i: RPC * i + nrows]
        xT = np.ascontiguousarray(
            xs.transpose(2, 0, 1).reshape(C, NKPIX)).astype(bf)
        in_maps.append({
            "xT": xT, "wqe": wqe_t, "wqo": wqo_t, "wk": wk_t, "wv": wv_t,
            "wo": wo_t, "mask": maskc,
        })

    nc = _get_nc()
    res = bass_utils.run_bass_kernel_spmd(nc, in_maps, core_ids=list(range(8)))
    out = np.empty((1, H, W, C), np.float32)
    for i in range(8):
        yTr = res.results[i]["yT"]
        out[0, RPC * i: RPC * (i + 1)] = (
            yTr.reshape(C, RPC, W).transpose(1, 2, 0))
    return out


# ---- fallback path (nonzero biases / bass-stack failure): jax pmap ----
def _kernel_jax(x, Wq, bq, Wk, bk, Wv, bv, Wo, bo):
    import jax
    import jax.numpy as jnp
    from functools import partial

    KRJ = RPC + 2 * R

    @partial(jax.pmap, in_axes=(0, None, None, None, None, None, None, None, None))
    def _shard_attn(xs, Wq, bq, Wk, bk, Wv, bv, Wo, bo):
        scale = 1.0 / np.sqrt(HD)
        Kp = xs @ Wk.T + bk
        Vp = xs @ Wv.T + bv
        center = xs[R:R + RPC, R:R + W, :]
        q = center @ Wq.T + bq
        Kw = jnp.stack([Kp[dy:dy + RPC, dx:dx + W, :]
                        for dy in range(WIN) for dx in range(WIN)], axis=2)
        Vw = jnp.stack([Vp[dy:dy + RPC, dx:dx + W, :]
                        for dy in range(WIN) for dx in range(WIN)], axis=2)
        qh = q.reshape(RPC, W, NH, HD)
        Kh = Kw.reshape(RPC, W, WIN * WIN, NH, HD)
        Vh = Vw.reshape(RPC, W, WIN * WIN, NH, HD)
        scores = jnp.einsum("xyhd,xywhd->xyhw", qh, Kh) * scale
        attn = jax.nn.softmax(scores, axis=-1)
        out = jnp.einsum("xyhw,xywhd->xyhd", attn, Vh).reshape(RPC, W, C)
        return out @ Wo.T + bo

    x = np.asarray(x, np.float32)
    xp = np.pad(x, ((0, 0), (R, R), (R, R), (0, 0)), mode="reflect")[0]
    shards = np.stack([xp[RPC * i: RPC * i + KRJ] for i in range(8)])
    out = _shard_attn(jnp.asarray(shards), *[jnp.asarray(np.asarray(a, np.float32))
          for a in (Wq, bq, Wk, bk, Wv, bv, Wo, bo)])
    return np.asarray(out).reshape(1, H, W, C).astype(np.float32)


def kernel(x, Wq, bq, Wk, bk, Wv, bv, Wo, bo):
    try:
        if any(np.any(np.asarray(b)) for b in (bq, bk, bv, bo)):
            return _kernel_jax(x, Wq, bq, Wk, bk, Wv, bv, Wo, bo)
        if np.asarray(x).shape != (1, H, W, C):
            return _kernel_jax(x, Wq, bq, Wk, bk, Wv, bv, Wo, bo)
        return _kernel_bass(x, Wq, bq, Wk, bk, Wv, bv, Wo, bo)
    except Exception:
        return _kernel_jax(x, Wq, bq, Wk, bk, Wv, bv, Wo, bo)


# revision 7
# speedup vs baseline: 12035.8075x; 12035.8075x over previous
"""Bass/Tile kernel for LocalWindowMultiHeadAttention on 8 trn2 cores.

v2: blocked transposed-score formulation. Per core (16 query rows):
  scores computed directly in [kpix, qx] orientation as S^T tiles
  [(4 krows x 32 kx), 26 qx] via K=32 matmuls (QE/QO even/odd-head channel
  masking), killing all P transposes. exp on ACT per half-row (4 heads x
  2 tiles x 128 qx), band-mask multiply on DVE/Pool, attn@V contracts the
  masked P tiles against V^T tiles (PE-transposed from the V projection)
  with a ones-column matmul for Z. Per-block normalize (scalar_tensor_tensor
  with 1/Z broadcast) into block-major attnN, per-block PE transposes
  reassemble [hd, qx], then the Wo projection.
"""

import sys
import numpy as np
import ml_dtypes
from contextlib import ExitStack

sys.path.insert(0, "/opt/trn_rl_repo")

import concourse.bass as bass
import concourse.mybir as mybir
import concourse.tile as tile
from concourse import bacc
from concourse.masks import make_identity
from concourse import bass_utils

BF16 = mybir.dt.bfloat16
F32 = mybir.dt.float32
ALU = mybir.AluOpType
AF = mybir.ActivationFunctionType

C = 128
NH = 8
HD = 16
R = 3
WIN = 7
H = W = 128
RPC = 16                  # query rows per core
KR = 23                   # padded k-rows per core (22 real + 1 pad)
KW = 136                  # padded k-cols (134 real + 2 pad)
NKPIX = KR * KW           # 3128
NQ = RPC * W              # 2048
SCALE = 1.0 / 4.0
NB = 5                    # qx blocks per row: 26,26,26,26,24
BW = [26, 26, 26, 26, 24]
BO = [0, 26, 52, 78, 104]
NS = 20                   # V^T tile starts (krow s..s+4)

_CACHE = {}


def build_nc():
    nc = bacc.Bacc()
    xT = nc.dram_tensor("xT", [C, NKPIX], BF16, kind="ExternalInput")
    wqe = nc.dram_tensor("wqe", [C, C], BF16, kind="ExternalInput")
    wqo = nc.dram_tensor("wqo", [C, C], BF16, kind="ExternalInput")
    wk = nc.dram_tensor("wk", [C, C], BF16, kind="ExternalInput")
    wv = nc.dram_tensor("wv", [C, C], BF16, kind="ExternalInput")
    wo = nc.dram_tensor("wo", [C, C], BF16, kind="ExternalInput")
    mask = nc.dram_tensor("mask", [C, 2 * 128], BF16, kind="ExternalInput")
    yT = nc.dram_tensor("yT", [C, NQ], F32, kind="ExternalOutput")

    with tile.TileContext(nc) as tc, ExitStack() as ctx:
        const = ctx.enter_context(tc.tile_pool(name="const", bufs=1))
        sb = ctx.enter_context(tc.tile_pool(name="sb", bufs=1))
        pbuf = ctx.enter_context(tc.tile_pool(name="pbuf", bufs=2))
        wrk = ctx.enter_context(tc.tile_pool(name="wrk", bufs=2))
        # PSUM: sc 2x2 banks, av 2 banks, fin 1 bank
        ps_sc = ctx.enter_context(tc.tile_pool(name="sc", bufs=2, space="PSUM"))
        ps_av = ctx.enter_context(tc.tile_pool(name="av", bufs=1, space="PSUM"))
        ps_fin = ctx.enter_context(tc.tile_pool(name="fin", bufs=1, space="PSUM"))

        ident = const.tile([128, 128], BF16)
        make_identity(nc, ident[:])
        ones_sb = const.tile([128, 1], BF16)
        nc.vector.memset(ones_sb[:], 1.0)

        xT_sb = const.tile([C, KR, KW], BF16)
        nc.sync.dma_start(xT_sb[:], xT[:].rearrange("c (r x) -> c r x", x=KW))
        wqe_sb = const.tile([C, C], BF16)
        nc.sync.dma_start(wqe_sb[:], wqe[:])
        wqo_sb = const.tile([C, C], BF16)
        nc.sync.dma_start(wqo_sb[:], wqo[:])
        wk_sb = const.tile([C, C], BF16)
        nc.sync.dma_start(wk_sb[:], wk[:])
        wv_sb = const.tile([C, C], BF16)
        nc.sync.dma_start(wv_sb[:], wv[:])
        wo_sb = const.tile([C, C], BF16)
        nc.sync.dma_start(wo_sb[:], wo[:])
        mask_sb = const.tile([C, 2, 128], BF16)
        nc.sync.dma_start(mask_sb[:], mask[:].rearrange("c (t q) -> c t q", q=128))

        KS = const.tile([C, KR, KW], BF16)
        VS = const.tile([C, KR, KW], BF16)
        QE = const.tile([C, RPC, W], BF16)
        QO = const.tile([C, RPC, W], BF16)
        VT = const.tile([128, NS, NB, 128], BF16)

        xflat = xT_sb[:].rearrange("c r x -> c (r x)")

        # ---- projections: K, V over all kpix; Q (even/odd) over centers ----
        # chunks of 1024 cols through the sc pool (2 banks each, bufs=2)
        def proj_copy(dst_flat, w_sb, src_flat, n0, n, eng):
            t = ps_sc.tile([128, 1024], F32, tag="sc")
            h1 = min(512, n)
            nc.tensor.matmul(t[:, 0:h1], w_sb[:], src_flat[:, n0:n0 + h1],
                             start=True, stop=True)
            if n > 512:
                nc.tensor.matmul(t[:, 512:512 + n - 512], w_sb[:],
                                 src_flat[:, n0 + 512:n0 + n],
                                 start=True, stop=True)
            if eng == "act":
                nc.scalar.copy(dst_flat[:, n0:n0 + n], t[:, 0:n])
            else:
                nc.vector.tensor_copy(dst_flat[:, n0:n0 + n], t[:, 0:n])

        KSf = KS[:].rearrange("c r x -> c (r x)")
        VSf = VS[:].rearrange("c r x -> c (r x)")
        QEf = QE[:].rearrange("c r x -> c (r x)")
        QOf = QO[:].rearrange("c r x -> c (r x)")
        def qproj(dst_flat, w_sb, j, eng):
            # rows 8j..8j+8, 1024 cols, via two 512-col matmuls (3-D rhs APs)
            t = ps_sc.tile([128, 1024], F32, tag="sc")
            for a in range(2):
                nc.tensor.matmul(t[:, 512 * a:512 * (a + 1)], w_sb[:],
                                 xT_sb[:, 3 + 8 * j + 4 * a:3 + 8 * j + 4 * a + 4,
                                       3:3 + W],
                                 start=True, stop=True)
            if eng == "act":
                nc.scalar.copy(dst_flat[:, 1024 * j:1024 * (j + 1)], t[:])
            else:
                nc.vector.tensor_copy(dst_flat[:, 1024 * j:1024 * (j + 1)], t[:])

        # interleave K and Q first (needed by scores of early rows), then V
        proj_copy(KSf, wk_sb, xflat, 0, 1024, "dve")
        qproj(QEf, wqe_sb, 0, "act")
        qproj(QOf, wqo_sb, 0, "act")
        proj_copy(KSf, wk_sb, xflat, 1024, 1024, "dve")
        qproj(QEf, wqe_sb, 1, "act")
        qproj(QOf, wqo_sb, 1, "act")
        proj_copy(KSf, wk_sb, xflat, 2048, 1024, "dve")
        proj_copy(KSf, wk_sb, xflat, 3072, NKPIX - 3072, "dve")
        for j in range(4):
            n0 = 1024 * j
            n = min(1024, NKPIX - n0)
            proj_copy(VSf, wv_sb, xflat, n0, n, "act" if j % 2 else "dve")

        # ---- V^T tiles via PE transpose of VS ----
        def emit_vt(s):
            vtp = ps_fin.tile([128, NB, 128], BF16, tag="vt")
            for b in range(NB):
                nc.tensor.transpose(vtp[:, b, :],
                                    VS[:, s:s + 4, BO[b]:BO[b] + 32],
                                    ident[:])
            nc.vector.tensor_copy(VT[:, s, :, :], vtp[:])

        for s in range(4):
            emit_vt(s)

        mask_bc = mask_sb[:].rearrange("c t q -> c () t q").broadcast_to(
            [C, 4, 2, 128])

        # ---- main pipeline over rows ----
        sc_tiles = {}
        PM_tiles = {}
        av_tiles = {}
        nb_tiles = {}
        atS_tiles = {}

        def emit_scores(r, half):
            t = ps_sc.tile([128, 1024], F32, tag="sc")
            sc_tiles[(r, half)] = t
            tv = t[:].rearrange("p (h t q) -> p h t q", h=4, t=2)
            for g01 in range(2):
                g = 2 * half + g01
                pb = 32 * g
                for b in range(NB):
                    for tt in range(2):
                        lhs = KS[pb:pb + 32, r + 4 * tt:r + 4 * tt + 4,
                                 BO[b]:BO[b] + 32]
                        for par in range(2):
                            h4 = 2 * g01 + par
                            qsrc_ = QE if par == 0 else QO
                            rhs = qsrc_[pb:pb + 32, r, BO[b]:BO[b] + BW[b]]
                            nc.tensor.matmul(
                                tv[:, h4, tt, BO[b]:BO[b] + BW[b]],
                                lhs, rhs, start=True, stop=True,
                                tile_position=(pb, 0))

        def emit_exp_mask(r, half):
            t = sc_tiles.pop((r, half))
            P = pbuf.tile([128, 4, 2, 128], BF16, tag=f"P{half}")
            nc.scalar.activation(P[:].rearrange("p h t q -> p (h t q)"),
                                 t[:], AF.Exp, scale=SCALE)
            PM = pbuf.tile([128, 4, 2, 128], BF16, tag=f"PM{half}")
            PM_tiles[(r, half)] = PM
            if half == 0:
                nc.vector.tensor_mul(PM[:], P[:], mask_bc)
            else:
                nc.gpsimd.tensor_mul(PM[:], P[:], mask_bc)

        def emit_attnv(r):
            av = ps_av.tile([26, NB * 128 + NB * 8], F32, tag="av")
            av_tiles[r] = av
            attpD = av[:, 0:NB * 128].rearrange("p (b h d) -> p b h d",
                                                b=NB, h=NH)
            for b in range(NB):
                for h in range(NH):
                    half, h4 = divmod(h, 4)
                    PM = PM_tiles[(r, half)]
                    zc = NB * 128 + b * NH + h
                    for tt in range(2):
                        lhs = PM[:, h4, tt, BO[b]:BO[b] + BW[b]]
                        nc.tensor.matmul(
                            attpD[0:BW[b], b, h, :], lhs,
                            VT[:, r + 4 * tt, b, HD * h:HD * h + HD],
                            start=(tt == 0), stop=(tt == 1))
                        nc.tensor.matmul(
                            av[0:BW[b], zc:zc + 1], lhs, ones_sb[:],
                            start=(tt == 0), stop=(tt == 1))
            PM_tiles.pop((r, 0))
            PM_tiles.pop((r, 1))

        def emit_norm(r):
            av = av_tiles.pop(r)
            attpD = av[:, 0:NB * 128].rearrange("p (b h d) -> p b h d",
                                                b=NB, h=NH)
            Zp = av[:, NB * 128:].rearrange("p (b h) -> p b h", b=NB)
            rz = wrk.tile([26, NB, NH], F32, tag="rz")
            nc.vector.reciprocal(rz[:], Zp)
            nb_t = wrk.tile([26, NB, NH, HD], BF16, tag="nb")
            nb_tiles[r] = nb_t
            rzb = rz[:].rearrange("p b h -> p b h ()").broadcast_to(
                [26, NB, NH, HD])
            nc.vector.scalar_tensor_tensor(nb_t[:], attpD, 1.0, rzb,
                                           ALU.mult, ALU.mult)

        def emit_final(r):
            nb_t = nb_tiles.pop(r)
            atp = ps_fin.tile([128, 128], BF16, tag="fin")
            for b in range(NB):
                nc.tensor.transpose(
                    atp[:, BO[b]:BO[b] + BW[b]],
                    nb_t[0:BW[b], b, :, :].rearrange("p h d -> p (h d)"),
                    ident[0:BW[b], 0:BW[b]])
            atS = wrk.tile([128, 128], BF16, tag="atS")
            nc.vector.tensor_copy(atS[:], atp[:])
            yp = ps_fin.tile([128, 128], F32, tag="fin")
            nc.tensor.matmul(yp[:], wo_sb[:], atS[:], start=True, stop=True)
            yS = wrk.tile([128, 128], F32, tag="yS")
            nc.scalar.copy(yS[:], yp[:])
            nc.sync.dma_start(yT[:, W * r:W * (r + 1)], yS[:])

        for r in range(RPC + 2):
            if r < RPC:
                emit_scores(r, 0)
                emit_exp_mask(r, 0)
                emit_scores(r, 1)
                emit_exp_mask(r, 1)
                if r + 4 < NS:
                    emit_vt(r + 4)
            if 1 <= r <= RPC:
                emit_attnv(r - 1)
                emit_norm(r - 1)
            if r >= 2:
                emit_final(r - 2)
    nc.compile()
    return nc


def _get_nc():
    if "nc" not in _CACHE:
        _CACHE["nc"] = build_nc()
    return _CACHE["nc"]


def _host_mask():
    m = np.zeros((128, 2, 128), np.float32)
    p = np.arange(128)
    kxl = p % 32
    qx = np.arange(128)
    b = np.minimum(qx // 26, 4)
    qxl = qx - 26 * b
    band = (qxl[None, :] <= kxl[:, None]) & (kxl[:, None] <= qxl[None, :] + 6)
    m[:, 0, :] = band
    m[:, 1, :] = band & (p < 96)[:, None]
    return np.ascontiguousarray(m.reshape(128, 256)).astype(ml_dtypes.bfloat16)


def _kernel_bass(x, Wq, bq, Wk, bk, Wv, bv, Wo, bo):
    x = np.asarray(x, np.float32)
    Wq, Wk, Wv, Wo = (np.asarray(w, np.float32) for w in (Wq, Wk, Wv, Wo))

    xp = np.pad(x, ((0, 0), (R, R), (R, R), (0, 0)), mode="reflect")[0]
    bf = ml_dtypes.bfloat16
    ch = np.arange(C)
    even = ((ch // HD) % 2 == 0).astype(np.float32)
    wqe_t = np.ascontiguousarray(Wq.T * even[None, :]).astype(bf)
    wqo_t = np.ascontiguousarray(Wq.T * (1.0 - even)[None, :]).astype(bf)
    wk_t = np.ascontiguousarray(Wk.T).astype(bf)
    wv_t = np.ascontiguousarray(Wv.T).astype(bf)
    wo_t = np.ascontiguousarray(Wo.T).astype(bf)
    maskc = _host_mask()

    in_maps = []
    for i in range(8):
        xs = np.zeros((KR, KW, C), np.float32)
        nrows = min(KR, 134 - RPC * i)
        xs[:nrows, :134] = xp[RPC * i: RPC * i + nrows]
        xT = np.ascontiguousarray(
            xs.transpose(2, 0, 1).reshape(C, NKPIX)).astype(bf)
        in_maps.append({
            "xT": xT, "wqe": wqe_t, "wqo": wqo_t, "wk": wk_t, "wv": wv_t,
            "wo": wo_t, "mask": maskc,
        })

    nc = _get_nc()
    res = bass_utils.run_bass_kernel_spmd(nc, in_maps, core_ids=list(range(8)))
    out = np.empty((1, H, W, C), np.float32)
    for i in range(8):
        yTr = res.results[i]["yT"]
        out[0, RPC * i: RPC * (i + 1)] = (
            yTr.reshape(C, RPC, W).transpose(1, 2, 0))
    return out


# ---- fallback path (nonzero biases / bass-stack failure): jax pmap ----
def _kernel_jax(x, Wq, bq, Wk, bk, Wv, bv, Wo, bo):
    import jax
    import jax.numpy as jnp
    from functools import partial

    KRJ = RPC + 2 * R

    @partial(jax.pmap, in_axes=(0, None, None, None, None, None, None, None, None))
    def _shard_attn(xs, Wq, bq, Wk, bk, Wv, bv, Wo, bo):
        scale = 1.0 / np.sqrt(HD)
        Kp = xs @ Wk.T + bk
        Vp = xs @ Wv.T + bv
        center = xs[R:R + RPC, R:R + W, :]
        q = center @ Wq.T + bq
        Kw = jnp.stack([Kp[dy:dy + RPC, dx:dx + W, :]
                        for dy in range(WIN) for dx in range(WIN)], axis=2)
        Vw = jnp.stack([Vp[dy:dy + RPC, dx:dx + W, :]
                        for dy in range(WIN) for dx in range(WIN)], axis=2)
        qh = q.reshape(RPC, W, NH, HD)
        Kh = Kw.reshape(RPC, W, WIN * WIN, NH, HD)
        Vh = Vw.reshape(RPC, W, WIN * WIN, NH, HD)
        scores = jnp.einsum("xyhd,xywhd->xyhw", qh, Kh) * scale
        attn = jax.nn.softmax(scores, axis=-1)
        out = jnp.einsum("xyhw,xywhd->xyhd", attn, Vh).reshape(RPC, W, C)
        return out @ Wo.T + bo

    x = np.asarray(x, np.float32)
    xp = np.pad(x, ((0, 0), (R, R), (R, R), (0, 0)), mode="reflect")[0]
    shards = np.stack([xp[RPC * i: RPC * i + KRJ] for i in range(8)])
    out = _shard_attn(jnp.asarray(shards), *[jnp.asarray(np.asarray(a, np.float32))
          for a in (Wq, bq, Wk, bk, Wv, bv, Wo, bo)])
    return np.asarray(out).reshape(1, H, W, C).astype(np.float32)


def kernel(x, Wq, bq, Wk, bk, Wv, bv, Wo, bo):
    try:
        if any(np.any(np.asarray(b)) for b in (bq, bk, bv, bo)):
            return _kernel_jax(x, Wq, bq, Wk, bk, Wv, bv, Wo, bo)
        if np.asarray(x).shape != (1, H, W, C):
            return _kernel_jax(x, Wq, bq, Wk, bk, Wv, bv, Wo, bo)
        return _kernel_bass(x, Wq, bq, Wk, bk, Wv, bv, Wo, bo)
    except Exception:
        return _kernel_jax(x, Wq, bq, Wk, bk, Wv, bv, Wo, bo)
